# revision 1
# baseline (speedup 1.0000x reference)
"""BiModal attention kernel for Trainium2 (8 NeuronCores, data-parallel over batch).

Per core (one batch b): x, y: [2048, 128] fp32.
  S = x @ y.T                    (float32r matmuls, [2048, 2048])
  E = exp(S)                     (unshifted; softmax is shift-invariant and
                                  |S| <~ 67 so exp stays in fp32/bf16 range)
  a1 = (E @ y) / rowsum(E) * x
  a2 = (E.T @ x) / colsum(E) * y
  out = concat([a1, a2], -1)     ([2048, 256])
"""
import sys

sys.path.insert(0, "/opt/trn_rl_repo")

import os
import numpy as np

import concourse.bass as bass
import concourse.mybir as mybir
import concourse.tile as tile
from concourse import bacc
from concourse.bass_utils import run_bass_kernel_spmd
from concourse.masks import make_identity

f32 = mybir.dt.float32
f32r = mybir.dt.float32r
bf16 = mybir.dt.bfloat16

B = 8
S = 2048
D = 128
P = 128
NB = S // P          # 16 row/col blocks of 128
NQ = S // 512        # 4 quarters of 512

_NC_CACHE = None
LAST_EXEC_NS = None


def _build_program(nc):
    x_d = nc.dram_tensor("x", [S, D], f32, kind="ExternalInput").ap()
    y_d = nc.dram_tensor("y", [S, D], f32, kind="ExternalInput").ap()
    out_d = nc.dram_tensor("out", [S, 2 * D], f32, kind="ExternalOutput").ap()

    x_dv = x_d.rearrange("(b p) d -> p b d", p=P)      # [128, 16, 128]
    y_dv = y_d.rearrange("(b p) d -> p b d", p=P)
    out_dv = out_d.rearrange("(b p) c -> p b c", p=P)  # [128, 16, 256]

    Exp = mybir.ActivationFunctionType.Exp
    MUL = mybir.AluOpType.mult
    ADD = mybir.AluOpType.add
    AX = mybir.AxisListType.X

    with tile.TileContext(nc) as tc:
        with (
            tc.tile_pool(name="sb", bufs=1) as sb,
            tc.tile_pool(name="stg", bufs=4) as stg,
            tc.tile_pool(name="ps", bufs=1, space="PSUM") as ps,
        ):
            # ---- persistent SBUF tensors ----
            x_sb = sb.tile([P, NB, D], f32, tag="x_sb")
            y_sb = sb.tile([P, NB, D], f32, tag="y_sb")
            xT = sb.tile([P, S], f32r, tag="xT")      # [d, s], f32r-rounded
            yT = sb.tile([P, S], f32r, tag="yT")      # [d, t]
            x_bf = sb.tile([P, NB, D], bf16, tag="x_bf")
            y_bf = sb.tile([P, NB, D], bf16, tag="y_bf")
            E = sb.tile([P, NB, S], bf16, tag="E")    # [sp, sb, t]
            ET = sb.tile([P, NB, S], bf16, tag="ET")  # [tp, tb, s]
            o1T_sb = sb.tile([P, S], f32, tag="o1T")  # [d, s]
            o2T_sb = sb.tile([P, S], f32, tag="o2T")  # [d, t]
            ident = sb.tile([P, P], f32, tag="ident")
            l1p = sb.tile([P, 2 * NB], f32, tag="l1p")    # [sp, 2*i+h]
            l2p = sb.tile([P, NB, NB], f32, tag="l2p")    # [tp, tb, i]
            l1 = sb.tile([P, NB], f32, tag="l1")
            l2 = sb.tile([P, NB], f32, tag="l2")
            r1 = sb.tile([P, NB], f32, tag="r1")
            r2 = sb.tile([P, NB], f32, tag="r2")

            # ---- loads ----
            nc.sync.dma_start(x_sb[:], x_dv)
            nc.sync.dma_start(y_sb[:], y_dv)
            make_identity(nc, ident[:])
            nc.vector.tensor_copy(x_bf[:], x_sb[:])
            nc.vector.tensor_copy(y_bf[:], y_sb[:])

            # ---- prologue: xT/yT via PE transpose + DVE round-to-f32r ----
            pro_ps = ps.tile([P, 4, 512], f32, tag="B")
            for k in range(NB):
                nc.tensor.transpose(pro_ps[:, k % 4, 0:P], y_sb[:, k, :], ident[:])
                nc.vector.tensor_copy(yT[:, k * P:(k + 1) * P], pro_ps[:, k % 4, 0:P])
            for k in range(NB):
                nc.tensor.transpose(pro_ps[:, k % 4, P:2 * P], x_sb[:, k, :], ident[:])
                nc.vector.tensor_copy(xT[:, k * P:(k + 1) * P], pro_ps[:, k % 4, P:2 * P])

            # ---- main pipeline over row blocks i ----
            s_ps = ps.tile([P, 2, 1024], f32, tag="A")   # S psum, 2x1024 halves
            o2_ps = ps.tile([P, 4, 512], f32, tag="B")   # o2T accumulator
            for i in range(NB):
                xti = xT[:, i * P:(i + 1) * P]
                for h in range(2):
                    nc.tensor.matmul(s_ps[:, h, 0:512], xti,
                                     yT[:, h * 1024:h * 1024 + 512],
                                     start=True, stop=True)
                    nc.tensor.matmul(s_ps[:, h, 512:1024], xti,
                                     yT[:, h * 1024 + 512:h * 1024 + 1024],
                                     start=True, stop=True)
                    nc.scalar.activation(E[:, i, h * 1024:(h + 1) * 1024],
                                         s_ps[:, h, :], Exp,
                                         accum_out=l1p[:, 2 * i + h:2 * i + h + 1])
                # o2T += x_bf[i].T @ E[i]
                for q in range(NQ):
                    nc.tensor.matmul(o2_ps[:, q, :], x_bf[:, i, :],
                                     E[:, i, q * 512:(q + 1) * 512],
                                     start=(i == 0), stop=(i == NB - 1))
                # transpose E row-block into ET columns
                nc.sync.dma_start_transpose(ET[:, :, i * P:(i + 1) * P], E[:, i, :])
                # column-sum partial (over this s-block) from ET
                nc.vector.tensor_reduce(l2p[:, :, i], ET[:, :, i * P:(i + 1) * P],
                                        axis=AX, op=ADD)

            # ---- normalizers ----
            nc.vector.tensor_reduce(l1[:], l1p[:].rearrange("p (i h) -> p i h", h=2),
                                    axis=AX, op=ADD)
            nc.vector.reciprocal(r1[:], l1[:])
            nc.vector.tensor_reduce(l2[:], l2p[:], axis=AX, op=ADD)
            nc.vector.reciprocal(r2[:], l2[:])

            # ---- drain o2T to SBUF (ACT), freeing PSUM tag B ----
            nc.scalar.copy(o2T_sb[:], o2_ps[:].rearrange("p a b -> p (a b)"))

            # ---- o1T = y_bf.T @ ET  (accumulate over t blocks) ----
            o1_ps = ps.tile([P, 4, 512], f32, tag="A")
            for tb in range(NB):
                for q in range(NQ):
                    nc.tensor.matmul(o1_ps[:, q, :], y_bf[:, tb, :],
                                     ET[:, tb, q * 512:(q + 1) * 512],
                                     start=(tb == 0), stop=(tb == NB - 1))
            nc.scalar.copy(o1T_sb[:], o1_ps[:].rearrange("p a b -> p (a b)"))

            # ---- epilogue: retranspose + gate + store, per row block j ----
            t2_ps = ps.tile([P, 4, 512], f32, tag="B")
            t1_ps = ps.tile([P, 4, 512], f32, tag="A")
            for j in range(NB):
                stage = stg.tile([P, 2 * D], f32, tag="stage")
                nc.tensor.transpose(t2_ps[:, j % 4, 0:P],
                                    o2T_sb[:, j * P:(j + 1) * P], ident[:])
                nc.vector.scalar_tensor_tensor(stage[:, D:2 * D],
                                               t2_ps[:, j % 4, 0:P],
                                               r2[:, j:j + 1],
                                               y_sb[:, j, :], op0=MUL, op1=MUL)
                nc.tensor.transpose(t1_ps[:, j % 4, 0:P],
                                    o1T_sb[:, j * P:(j + 1) * P], ident[:])
                nc.vector.scalar_tensor_tensor(stage[:, 0:D],
                                               t1_ps[:, j % 4, 0:P],
                                               r1[:, j:j + 1],
                                               x_sb[:, j, :], op0=MUL, op1=MUL)
                nc.sync.dma_start(out_dv[:, j, :], stage[:])

    nc.compile()
    return nc


def _get_nc():
    global _NC_CACHE
    if _NC_CACHE is None:
        nc = bacc.Bacc("TRN2", target_bir_lowering=False, debug=False,
                       num_devices=B)
        _NC_CACHE = _build_program(nc)
    return _NC_CACHE


def kernel(x, y):
    global LAST_EXEC_NS
    nc = _get_nc()
    x = np.asarray(x, dtype=np.float32)
    y = np.asarray(y, dtype=np.float32)
    in_maps = [
        {"x": np.ascontiguousarray(x[b]), "y": np.ascontiguousarray(y[b])}
        for b in range(B)
    ]
    trace = bool(int(os.environ.get("KERNEL_TRACE", "0")))
    res = run_bass_kernel_spmd(nc, in_maps, list(range(B)), trace=trace)
    LAST_EXEC_NS = res.exec_time_ns
    return np.stack([res.results[b]["out"] for b in range(B)], axis=0)


# revision 2
# speedup vs baseline: 1.0665x; 1.0665x over previous
"""BiModal attention kernel for Trainium2 (8 NeuronCores, data-parallel over batch).

Per core (one batch b): x, y: [2048, 128] fp32.
  S = x @ y.T                    (float32r matmuls, [2048, 2048])
  E = exp(S)                     (unshifted; softmax is shift-invariant and
                                  |S| <~ 67 so exp stays in fp32/bf16 range)
  a1 = (E @ y) / rowsum(E) * x
  a2 = (E.T @ x) / colsum(E) * y
  out = concat([a1, a2], -1)     ([2048, 256])

Layout: rows are relabeled s = 16*p + b (p = SBUF partition, b = block index)
so every DRAM transfer is contiguous per partition. The same relabeling is
applied consistently to s and t on all intermediate tensors, so the math is
unchanged.

Schedule (per core): two 1024-wide column panels. Per panel: S matmuls
(f32r) -> exp (ACT, PSUM->SBUF bf16, fused row-sum accumulation) -> xbar
DMA-transpose of E into ET -> DVE column-sum partials; o1T chunks (contract
over t) interleave one panel behind; o2T chunks + o1T tail + both epilogues
(PE retranspose + fused gate/normalize on DVE) form the final phase.
"""
import sys

sys.path.insert(0, "/opt/trn_rl_repo")

import os
import numpy as np

import concourse.bass as bass
import concourse.mybir as mybir
import concourse.tile as tile
from concourse import bacc
from concourse.bass_utils import run_bass_kernel_spmd
from concourse.masks import make_identity

f32 = mybir.dt.float32
f32r = mybir.dt.float32r
bf16 = mybir.dt.bfloat16

B = 8
S = 2048
D = 128
P = 128
NB = S // P          # 16 blocks
NP = 2               # panels
PW = S // NP         # panel width (1024)
PB = PW // P         # blocks per panel (8)

_NC_CACHE = None
LAST_EXEC_NS = None


def _build_program(nc):
    x_d = nc.dram_tensor("x", [S, D], f32, kind="ExternalInput").ap()
    y_d = nc.dram_tensor("y", [S, D], f32, kind="ExternalInput").ap()
    out_d = nc.dram_tensor("out", [S, 2 * D], f32, kind="ExternalOutput").ap()

    # contiguous-per-partition views; row r = 16*p + b
    x_dv = x_d.rearrange("(p b) d -> p b d", p=P)      # [128, 16, 128]
    y_dv = y_d.rearrange("(p b) d -> p b d", p=P)
    out_dv = out_d.rearrange("(p b) c -> p b c", p=P)  # [128, 16, 256]

    Exp = mybir.ActivationFunctionType.Exp
    MUL = mybir.AluOpType.mult
    ADD = mybir.AluOpType.add
    AX = mybir.AxisListType.X

    with tile.TileContext(nc) as tc:
        with (
            tc.tile_pool(name="sb", bufs=1) as sb,
            tc.tile_pool(name="stg", bufs=4) as stg,
            tc.tile_pool(name="ps", bufs=1, space="PSUM") as ps,
        ):
            # ---- persistent SBUF tensors ----
            x_sb = sb.tile([P, NB, D], f32, tag="x_sb")
            y_sb = sb.tile([P, NB, D], f32, tag="y_sb")
            xT = sb.tile([P, S], f32r, tag="xT")      # [d, s-pos]
            yT = sb.tile([P, S], f32r, tag="yT")      # [d, t-pos]
            x_bf = sb.tile([P, NB, D], bf16, tag="x_bf")
            y_bf = sb.tile([P, NB, D], bf16, tag="y_bf")
            E = sb.tile([P, NB, S], bf16, tag="E")    # [sp, sb, t-pos]
            ET = sb.tile([P, NB, S], bf16, tag="ET")  # [tp, tb, s-pos]
            o1T_sb = sb.tile([P, S], f32, tag="o1T")  # [d, s-pos]
            o2T_sb = sb.tile([P, S], f32, tag="o2T")  # [d, t-pos]
            ident = sb.tile([P, P], f32, tag="ident")
            l1p = sb.tile([P, 2 * NB], f32, tag="l1p")    # [sp, 2*i+ct]
            l2p = sb.tile([P, NB, NB], f32, tag="l2p")    # [tp, tb, i]
            l1 = sb.tile([P, NB], f32, tag="l1")
            l2 = sb.tile([P, NB], f32, tag="l2")
            r1 = sb.tile([P, NB], f32, tag="r1")
            r2 = sb.tile([P, NB], f32, tag="r2")

            # ---- loads ----
            make_identity(nc, ident[:])
            nc.sync.dma_start(y_sb[:], y_dv)
            nc.sync.dma_start(x_sb[:], x_dv)

            # ---- prologue: xT/yT via PE transpose + DVE round-to-f32r ----
            pro_ps = ps.tile([P, 4, 512], f32, tag="B")

            def pro_transpose(src_sb, dst, k, slot):
                nc.tensor.transpose(pro_ps[:, slot % 4, 0:P], src_sb[:, k, :],
                                    ident[:])
                nc.vector.tensor_copy(dst[:, k * P:(k + 1) * P],
                                      pro_ps[:, slot % 4, 0:P])

            for k in range(PB):                      # y blocks 0..7 (panel 0)
                pro_transpose(y_sb, yT, k, k)
            for k in range(NB):                      # all x blocks
                pro_transpose(x_sb, xT, k, PB + k)
            for k in range(PB, NB):                  # y blocks 8..15 (panel 1)
                pro_transpose(y_sb, yT, k, PB + NB + k)

            # casts for the bf16 matmuls (needed from phase o1T/o2T onwards)
            nc.vector.tensor_copy(y_bf[:], y_sb[:])
            nc.vector.tensor_copy(x_bf[:], x_sb[:])

            # ---- main: panels of 1024 columns ----
            s_ps = ps.tile([P, 2, PW], f32, tag="A")     # S psum, 2-deep rotation
            o1_ps = ps.tile([P, 4, 512], f32, tag="B")   # o1T accumulator

            def o1_chunk(tb):
                # o1T[:, q] += y_bf[tb].T @ ET[tb, q]   (contract 128 t's)
                for q in range(4):
                    nc.tensor.matmul(o1_ps[:, q, :], y_bf[:, tb, :],
                                     ET[:, tb, q * 512:(q + 1) * 512],
                                     start=(tb == 0), stop=(tb == NB - 1))

            for ct in range(NP):
                c0 = ct * PW
                for i in range(NB):
                    xti = xT[:, i * P:(i + 1) * P]
                    slot = s_ps[:, i % 2, :]
                    nc.tensor.matmul(slot[:, 0:512], xti,
                                     yT[:, c0:c0 + 512], start=True, stop=True)
                    nc.tensor.matmul(slot[:, 512:1024], xti,
                                     yT[:, c0 + 512:c0 + 1024],
                                     start=True, stop=True)
                    # interleave one o1T chunk of the previous panel
                    if ct > 0 and i < PB:
                        o1_chunk((ct - 1) * PB + i)
                    nc.scalar.activation(E[:, i, c0:c0 + PW], slot, Exp,
                                         accum_out=l1p[:, 2 * i + ct:2 * i + ct + 1])
                    nc.sync.dma_start_transpose(
                        ET[:, ct * PB:(ct + 1) * PB, i * P:(i + 1) * P],
                        E[:, i, c0:c0 + PW])
                    nc.vector.tensor_reduce(
                        l2p[:, ct * PB:(ct + 1) * PB, i],
                        ET[:, ct * PB:(ct + 1) * PB, i * P:(i + 1) * P],
                        axis=AX, op=ADD)

            # ---- normalizers ----
            nc.vector.tensor_reduce(l1[:], l1p[:].rearrange("p (i c) -> p i c", c=2),
                                    axis=AX, op=ADD)
            nc.vector.reciprocal(r1[:], l1[:])
            nc.vector.tensor_reduce(l2[:], l2p[:], axis=AX, op=ADD)
            nc.vector.reciprocal(r2[:], l2[:])

            # ---- final phase: o2T chunks + trailing o1T chunks ----
            o2_ps = ps.tile([P, 4, 512], f32, tag="A")
            for i in range(NB):
                for q in range(4):
                    nc.tensor.matmul(o2_ps[:, q, :], x_bf[:, i, :],
                                     E[:, i, q * 512:(q + 1) * 512],
                                     start=(i == 0), stop=(i == NB - 1))
                if i < PB:
                    o1_chunk((NP - 1) * PB + i)

            nc.scalar.copy(o1T_sb[:], o1_ps[:].rearrange("p a b -> p (a b)"))

            # ---- epilogue 1: a1 = o1 * x * r1 ----
            e1_ps = ps.tile([P, 4, 512], f32, tag="B")
            for j in range(NB):
                st1 = stg.tile([P, D], f32, tag="st1")
                nc.tensor.transpose(e1_ps[:, j % 4, 0:P],
                                    o1T_sb[:, j * P:(j + 1) * P], ident[:])
                nc.vector.scalar_tensor_tensor(st1[:], e1_ps[:, j % 4, 0:P],
                                               r1[:, j:j + 1], x_sb[:, j, :],
                                               op0=MUL, op1=MUL)
                nc.sync.dma_start(out_dv[:, j, 0:D], st1[:])

            nc.scalar.copy(o2T_sb[:], o2_ps[:].rearrange("p a b -> p (a b)"))

            # ---- epilogue 2: a2 = o2 * y * r2 ----
            e2_ps = ps.tile([P, 4, 512], f32, tag="A")
            for j in range(NB):
                st2 = stg.tile([P, D], f32, tag="st2")
                nc.tensor.transpose(e2_ps[:, j % 4, 0:P],
                                    o2T_sb[:, j * P:(j + 1) * P], ident[:])
                nc.vector.scalar_tensor_tensor(st2[:], e2_ps[:, j % 4, 0:P],
                                               r2[:, j:j + 1], y_sb[:, j, :],
                                               op0=MUL, op1=MUL)
                nc.sync.dma_start(out_dv[:, j, D:2 * D], st2[:])

    nc.compile()
    return nc


def _get_nc():
    global _NC_CACHE
    if _NC_CACHE is None:
        nc = bacc.Bacc("TRN2", target_bir_lowering=False, debug=False,
                       num_devices=B)
        _NC_CACHE = _build_program(nc)
    return _NC_CACHE


def kernel(x, y):
    global LAST_EXEC_NS
    nc = _get_nc()
    x = np.asarray(x, dtype=np.float32)
    y = np.asarray(y, dtype=np.float32)
    in_maps = [
        {"x": np.ascontiguousarray(x[b]), "y": np.ascontiguousarray(y[b])}
        for b in range(B)
    ]
    trace = bool(int(os.environ.get("KERNEL_TRACE", "0")))
    res = run_bass_kernel_spmd(nc, in_maps, list(range(B)), trace=trace)
    LAST_EXEC_NS = res.exec_time_ns
    return np.stack([res.results[b]["out"] for b in range(B)], axis=0)
